# revision 5
# baseline (speedup 1.0000x reference)
"""Trainium2 Bass kernel for nn_EntityCell (scatter_memory).

Math (per batch row b, entity e):
    gates = sigmoid(sum_d(s * (h + k)))              [B, E]
    h_tilda = tanh(h @ U + k @ V + (s @ W)[:, None]) [B, E, D]
    updated = h + gates[:, :, None] * h_tilda
    out = updated / sqrt(max(sum_d(updated^2), 1e-12))

Sharding: pure data parallel over the batch dim across 8 NeuronCores.

Per-core dataflow (B_loc=1024 rows; 4 load-pairs of 256 rows, each processed
as two 128-row compute chunks). v2: ROW-MAJOR pipeline. The matmuls run with
the transposed state tiles as the PE *stationary* operand (lhsT = hT_e /
kT_e / sT per entity) and U/V/W as the *moving* operand, so the PSUM result
is row-major [rows, f] and everything downstream (update, norm, scale,
store) stays row-major — no transpose back, no full-width sigmoid:

  - SWDGE (gpsimd) DMA loads cast fp32 HBM -> fp16 SBUF row-major.
  - hT/kT via HWDGE xbar DMA transposes on the SP queue (zero compute-engine
    cost); sT via tiny PE transpose + ACT evac.
  - Gates (transposed domain, compact result): t2h = hT*sT_b, t2k = kT*sT_b
    (DVE TTs, middle-dim broadcast keeps the 2x mode), then per-entity PE
    ones-matmuls accumulate gpp[rows, e] = sum_d(t2h_e + t2k_e) in PSUM;
    ACT sigmoid -> g16 [rows, E] fp16 (compact).
  - Matmuls per entity: ps_e[rows, f] = hT_e^T@U + kT_e^T@V + sT^T@W in
    PSUM; ACT tanh evac -> h~ row-major fp16.
  - Update: m = g_b*h~ (Pool TT; inner-dim broadcast is free there),
    u = m + h (DVE TT).
  - Norm: u2 = u*u (Pool TT), nsq = DVE tensor_reduce(X) -> [rows, E] f32;
    per-pair epilogue: bit-trick rsqrt + 1 Newton step (DVE smalls; avoids
    ACT table thrash) -> y fp32, y16 fp16.
  - Scale: o = u*y_b split across DVE / Pool / ACT (ACT per-entity Copy
    with per-partition scale=y) ; store one SWDGE DMA per 256-row pair,
    fp16 -> fp32 upcast.

Engine busy time sits well below the ~90us HBM roofline so the DMA wall
dominates.
"""

import numpy as np
from contextlib import nullcontext as _nullctx

B, E, D = 8192, 20, 128
N_CORES = 8
B_LOC = B // N_CORES
CHUNK = 128
N_PAIRS = B_LOC // (2 * CHUNK)
EG = 4  # entities per psum group (4*128 fp32 = one 2KB PSUM bank)
NG = E // EG

_CACHE = {}


def _build_nc(reps=1, loop_n=None, ablate='full',
              io_bufs=3, tr_bufs=2, trs_bufs=2, tt_bufs=2, bf_bufs=2,
              u2_bufs=2, sm_bufs=4, o_bufs=2,
              psm_bufs=3, psk_bufs=2, psg_bufs=2,
              m_eng='gps', u2_eng='gps', u_eng='dve',
              scale_dve_e=6, scale_gps_e=6,
              s_pe=True, split_first=1, store_pair=True, newton_iters=1,
              dma_scratch=32768):
    import concourse.tile as tile
    from concourse import bacc, mybir
    from concourse.masks import make_identity
    from contextlib import ExitStack

    fp32 = mybir.dt.float32
    fp16 = mybir.dt.float16
    int32 = mybir.dt.int32
    AF = mybir.ActivationFunctionType
    OP = mybir.AluOpType
    AX = mybir.AxisListType

    nc = bacc.Bacc("TRN2", target_bir_lowering=False, debug=False,
                   dynamic_dma_scratch_size=dma_scratch)
    enc_d = nc.declare_dram_parameter("enc", [B_LOC, D], fp32, isOutput=False)
    prev_d = nc.declare_dram_parameter("prev", [B_LOC, E, D], fp32, isOutput=False)
    keys_d = nc.declare_dram_parameter("keys", [B_LOC, E, D], fp32, isOutput=False)
    u_d = nc.declare_dram_parameter("U", [D, D], fp32, isOutput=False)
    v_d = nc.declare_dram_parameter("V", [D, D], fp32, isOutput=False)
    w_d = nc.declare_dram_parameter("W", [D, D], fp32, isOutput=False)
    out_d = nc.declare_dram_parameter("out", [B_LOC, E, D], fp32, isOutput=True)

    prev_v = prev_d[:].rearrange("(n two p) e d -> n p two (e d)", two=2, p=CHUNK)
    keys_v = keys_d[:].rearrange("(n two p) e d -> n p two (e d)", two=2, p=CHUNK)
    enc_v = enc_d[:].rearrange("(n two p) d -> n p two d", two=2, p=CHUNK)
    out_pv = out_d[:].rearrange("(n two p) e d -> n p two (e d)", two=2, p=CHUNK)
    out_cv = out_d[:].rearrange("(n p) e d -> n p (e d)", p=CHUNK)

    sd, sp = scale_dve_e, scale_gps_e  # entity split for the final scale

    with ExitStack() as ctx:
        tc = ctx.enter_context(tile.TileContext(nc))
        const_pool = ctx.enter_context(tc.tile_pool(name="const", bufs=1))
        io_pool = ctx.enter_context(tc.tile_pool(name="io", bufs=io_bufs))
        tr_pool = ctx.enter_context(tc.tile_pool(name="tr", bufs=tr_bufs))
        trs_pool = ctx.enter_context(tc.tile_pool(name="trs", bufs=trs_bufs))
        tt_pool = ctx.enter_context(tc.tile_pool(name="tt", bufs=tt_bufs))
        bf_pool = ctx.enter_context(tc.tile_pool(name="bf", bufs=bf_bufs))
        u2_pool = ctx.enter_context(tc.tile_pool(name="u2", bufs=u2_bufs))
        sm_pool = ctx.enter_context(tc.tile_pool(name="sm", bufs=sm_bufs))
        o_pool = ctx.enter_context(tc.tile_pool(name="o", bufs=o_bufs))
        psm_pool = ctx.enter_context(
            tc.tile_pool(name="psm", bufs=psm_bufs, space="PSUM"))
        psk_pool = ctx.enter_context(
            tc.tile_pool(name="psk", bufs=psk_bufs, space="PSUM"))
        psg_pool = ctx.enter_context(
            tc.tile_pool(name="psg", bufs=psg_bufs, space="PSUM"))

        # ---- constants ----
        u32c = const_pool.tile([D, D], fp32)
        v32c = const_pool.tile([D, D], fp32)
        w32c = const_pool.tile([D, D], fp32)
        nc.sync.dma_start(u32c[:], u_d[:])
        nc.sync.dma_start(v32c[:], v_d[:])
        nc.sync.dma_start(w32c[:], w_d[:])
        u16c = const_pool.tile([D, D], fp16)
        v16c = const_pool.tile([D, D], fp16)
        w16c = const_pool.tile([D, D], fp16)
        nc.scalar.copy(u16c[:], u32c[:])
        nc.scalar.copy(v16c[:], v32c[:])
        nc.scalar.copy(w16c[:], w32c[:])
        ones1 = const_pool.tile([D, 1], fp16)
        nc.vector.memset(ones1[:], 1.0)
        magic2 = const_pool.tile([CHUNK, 2, E], int32)
        nc.vector.memset(magic2[:], 0x5F3759DF)
        ident16 = const_pool.tile([D, D], fp16)

        if loop_n is not None:
            make_identity(nc, ident16[:])
        loop_cm = (
            tc.For_i(0, loop_n, 1, hint_engines=tuple(mybir.ALL_ENGINES))
            if loop_n is not None
            else _nullctx()
        )
        ident_made = loop_n is not None
        with loop_cm:
         for cp in range(N_PAIRS * reps):
            n = cp % N_PAIRS
            # ---- paired loads (256 rows), SWDGE cast fp32 -> fp16 ----
            h16p = io_pool.tile([CHUNK, 2, E, D], fp16, name="h16p")
            k16p = io_pool.tile([CHUNK, 2, E, D], fp16, name="k16p")
            s16p = io_pool.tile([CHUNK, 2, D], fp16, name="s16p")
            if split_first and cp < int(split_first):
                eh = E // 2
                for hh in range(2):
                    if cp == 0 and hh == 0:
                        # entity-split first chunk so the first xbars start
                        # after a quarter of the pair load
                        for t_, v_ in ((h16p, prev_v), (k16p, keys_v)):
                            nc.gpsimd.dma_start(
                                t_[:, 0, :eh].rearrange("p e d -> p (e d)"),
                                v_[n][:, 0, : eh * D])
                            nc.gpsimd.dma_start(
                                t_[:, 0, eh:].rearrange("p e d -> p (e d)"),
                                v_[n][:, 0, eh * D :])
                    else:
                        nc.gpsimd.dma_start(
                            h16p[:, hh].rearrange("p e d -> p (e d)"),
                            prev_v[n][:, hh])
                        nc.gpsimd.dma_start(
                            k16p[:, hh].rearrange("p e d -> p (e d)"),
                            keys_v[n][:, hh])
            else:
                nc.gpsimd.dma_start(
                    h16p[:].rearrange("p a e d -> p a (e d)"), prev_v[n])
                nc.gpsimd.dma_start(
                    k16p[:].rearrange("p a e d -> p a (e d)"), keys_v[n])
            nc.gpsimd.dma_start(s16p[:], enc_v[n])
            if not ident_made:
                make_identity(nc, ident16[:])
                ident_made = True

            if ablate == 'dma':
                hb = h16p[:].rearrange("p a e d -> p a (e d)").bitcast(fp32)
                kb = k16p[:].rearrange("p a e d -> p a (e d)").bitcast(fp32)
                nc.gpsimd.dma_start(out=out_pv[n][:, :, : E * D // 2], in_=hb)
                nc.gpsimd.dma_start(out=out_pv[n][:, :, E * D // 2 :], in_=kb)
                continue

            nsq = sm_pool.tile([CHUNK, 2, E], fp32, name="nsq")
            o16p = o_pool.tile([CHUNK, 2, E, D], fp16, name="o16p")
            for half in range(2):
                c = 2 * n + half
                h16 = h16p[:, half]
                k16 = k16p[:, half]
                s16 = s16p[:, half]

                # ---- forward transposes: h/k xbars on the SP HWDGE queue,
                # s via tiny PE transpose + ACT evac ----
                hT = tr_pool.tile([D, E, CHUNK], fp16, name="hT")
                kT = tr_pool.tile([D, E, CHUNK], fp16, name="kT")
                sT = trs_pool.tile([D, CHUNK], fp16, name="sT")
                nc.sync.dma_start(out=hT[:], in_=h16, transpose=True)
                nc.sync.dma_start(out=kT[:], in_=k16, transpose=True)
                if s_pe:
                    stp = psk_pool.tile([D, CHUNK], fp16, name="stp", tag="tp")
                    nc.tensor.transpose(stp[:], s16, ident16[:])
                    nc.scalar.copy(sT[:], stp[:])
                else:
                    nc.sync.dma_start(out=sT[:], in_=s16, transpose=True)

                # ---- gates: t2h/t2k DVE TTs (2x mode: middle-dim bcast),
                # per-entity PE ones-reduce -> compact sigmoid ----
                sTb = sT[:].unsqueeze(1).broadcast_to([D, E, CHUNK])
                t2h = tt_pool.tile([D, E, CHUNK], fp16, name="t2h")
                t2k = tt_pool.tile([D, E, CHUNK], fp16, name="t2k")
                nc.vector.tensor_tensor(t2h[:], hT[:], sTb, OP.mult)
                nc.vector.tensor_tensor(t2k[:], kT[:], sTb, OP.mult)
                gpp = psg_pool.tile([CHUNK, E], fp32, name="gpp")
                for e in range(E):
                    nc.tensor.matmul(
                        gpp[:, e : e + 1], t2h[:, e], ones1[:],
                        start=True, stop=False)
                    nc.tensor.matmul(
                        gpp[:, e : e + 1], t2k[:, e], ones1[:],
                        start=False, stop=True)
                g16 = sm_pool.tile([CHUNK, E], fp16, name="g16")
                nc.scalar.activation(g16[:], gpp[:], AF.Sigmoid)

                # ---- matmuls (row-major psum) + tanh ----
                htil = bf_pool.tile([CHUNK, E, D], fp16, name="htil")
                for gi in range(NG):
                    ps = psm_pool.tile([CHUNK, EG, D], fp32, name="ps")
                    for j in range(EG):
                        e = gi * EG + j
                        nc.tensor.matmul(
                            ps[:, j], hT[:, e], u16c[:],
                            start=True, stop=False)
                        nc.tensor.matmul(
                            ps[:, j], kT[:, e], v16c[:],
                            start=False, stop=False)
                        nc.tensor.matmul(
                            ps[:, j], sT[:], w16c[:],
                            start=False, stop=True)
                    nc.scalar.activation(
                        htil[:, gi * EG : (gi + 1) * EG], ps[:], AF.Tanh)

                # ---- update: u = h + g*h~ (m on Pool, add on DVE) ----
                gb = g16[:].unsqueeze(2).broadcast_to([CHUNK, E, D])
                m_ = htil  # in-place over the tanh output
                if m_eng == 'gps':
                    nc.gpsimd.tensor_tensor(m_[:], htil[:], gb, OP.mult)
                else:
                    nc.vector.tensor_tensor(m_[:], htil[:], gb, OP.mult)
                u16 = bf_pool.tile([CHUNK, E, D], fp16, name="u16")
                if u_eng == 'dve':
                    nc.vector.tensor_tensor(u16[:], m_[:], h16, OP.add)
                else:
                    nc.gpsimd.tensor_tensor(u16[:], m_[:], h16, OP.add)

                # ---- norm: nsq[:, half] = sum_d u^2 ----
                u2 = u2_pool.tile([CHUNK, E, D], fp16, name="u2")
                if u2_eng == 'gps':
                    nc.gpsimd.tensor_tensor(u2[:], u16[:], u16[:], OP.mult)
                else:
                    nc.vector.tensor_tensor(u2[:], u16[:], u16[:], OP.mult)
                nc.vector.tensor_reduce(nsq[:, half], u2[:], AX.X, OP.add)

                if half == 0:
                    u16_prev = u16
                    continue

                # ---- pair epilogue: y = rsqrt(nsq), bit-trick + Newton ----
                ti = sm_pool.tile([CHUNK, 2, E], int32, name="ti")
                nc.vector.tensor_scalar(
                    ti[:], nsq[:].bitcast(int32), 1, None,
                    op0=OP.logical_shift_right)
                yi = sm_pool.tile([CHUNK, 2, E], int32, name="yi")
                nc.vector.tensor_tensor(yi[:], magic2[:], ti[:], OP.subtract)
                y = yi[:].bitcast(fp32)
                for _ in range(newton_iters):
                    y2 = sm_pool.tile([CHUNK, 2, E], fp32, name="y2")
                    nc.vector.tensor_tensor(y2[:], y, y, OP.mult)
                    tt_ = sm_pool.tile([CHUNK, 2, E], fp32, name="tt_")
                    nc.vector.tensor_tensor(tt_[:], nsq[:], y2[:], OP.mult)
                    ww = sm_pool.tile([CHUNK, 2, E], fp32, name="ww")
                    nc.vector.tensor_scalar(
                        ww[:], tt_[:], -0.5, 1.5, op0=OP.mult, op1=OP.add)
                    yn = sm_pool.tile([CHUNK, 2, E], fp32, name="yn")
                    nc.vector.tensor_tensor(yn[:], y, ww[:], OP.mult)
                    y = yn[:]
                y16 = sm_pool.tile([CHUNK, 2, E], fp16, name="y16")
                nc.scalar.copy(y16[:], y)

                # ---- scale o = u*y_b (split DVE / Pool / ACT) + pair store
                for hh, u_c in ((0, u16_prev), (1, u16)):
                    if sd > 0:
                        yb = (y16[:, hh, :sd].unsqueeze(2)
                              .broadcast_to([CHUNK, sd, D]))
                        nc.vector.tensor_tensor(
                            o16p[:, hh, :sd], u_c[:, :sd], yb, OP.mult)
                    if sp > 0:
                        yb = (y16[:, hh, sd : sd + sp].unsqueeze(2)
                              .broadcast_to([CHUNK, sp, D]))
                        nc.gpsimd.tensor_tensor(
                            o16p[:, hh, sd : sd + sp], u_c[:, sd : sd + sp],
                            yb, OP.mult)
                    for e in range(sd + sp, E):
                        nc.scalar.activation(
                            o16p[:, hh, e], u_c[:, e], AF.Copy,
                            scale=y[:, hh, e : e + 1])
                if store_pair:
                    nc.gpsimd.dma_start(
                        out=out_pv[n],
                        in_=o16p[:].rearrange("p a e d -> p a (e d)"))
                else:
                    for hh in range(2):
                        nc.gpsimd.dma_start(
                            out=out_cv[2 * n + hh],
                            in_=o16p[:, hh].rearrange("p e d -> p (e d)"))

    nc.compile()
    return nc


def _get_nc():
    if "nc" not in _CACHE:
        _CACHE["nc"] = _build_nc()
    return _CACHE["nc"]


def kernel(encoded_sents, prev_states, keys, U, V, W):
    import sys

    if "/opt/trn_rl_repo" not in sys.path:
        sys.path.insert(0, "/opt/trn_rl_repo")
    from concourse.bass_utils import run_bass_kernel_spmd

    nc = _get_nc()
    enc = np.ascontiguousarray(np.asarray(encoded_sents, dtype=np.float32))
    prev = np.ascontiguousarray(np.asarray(prev_states, dtype=np.float32))
    kys = np.ascontiguousarray(np.asarray(keys, dtype=np.float32))
    U = np.ascontiguousarray(np.asarray(U, dtype=np.float32))
    V = np.ascontiguousarray(np.asarray(V, dtype=np.float32))
    W = np.ascontiguousarray(np.asarray(W, dtype=np.float32))

    in_maps = []
    for i in range(N_CORES):
        lo, hi = i * B_LOC, (i + 1) * B_LOC
        in_maps.append(
            {
                "enc": enc[lo:hi],
                "prev": prev[lo:hi],
                "keys": kys[lo:hi],
                "U": U,
                "V": V,
                "W": W,
            }
        )

    res = run_bass_kernel_spmd(nc, in_maps, list(range(N_CORES)))
    out = np.concatenate([res.results[i]["out"] for i in range(N_CORES)], axis=0)
    return out.astype(np.float32)


# revision 12
# speedup vs baseline: 1.1692x; 1.1692x over previous
"""Trainium2 Bass kernel for nn_EntityCell (scatter_memory).

Math (per batch row b, entity e):
    gates = sigmoid(sum_d(s * (h + k)))              [B, E]
    h_tilda = tanh(h @ U + k @ V + (s @ W)[:, None]) [B, E, D]
    updated = h + gates[:, :, None] * h_tilda
    out = updated / sqrt(max(sum_d(updated^2), 1e-12))

Sharding: pure data parallel over the batch dim across 8 NeuronCores.

Per-core dataflow (B_loc=1024 rows; 4 load-pairs of 256 rows, each processed
as two 128-row compute chunks). v2: ROW-MAJOR pipeline. The matmuls run with
the transposed state tiles as the PE *stationary* operand (lhsT = hT_e /
kT_e / sT per entity) and U/V/W as the *moving* operand, so the PSUM result
is row-major [rows, f] and everything downstream (update, norm, scale,
store) stays row-major — no transpose back, no full-width sigmoid:

  - SWDGE (gpsimd) DMA loads cast fp32 HBM -> fp16 SBUF row-major.
  - hT/kT via HWDGE xbar DMA transposes on the SP queue (zero compute-engine
    cost); sT via tiny PE transpose + ACT evac.
  - Gates (transposed domain, compact result): t2h = hT*sT_b, t2k = kT*sT_b
    (DVE TTs, middle-dim broadcast keeps the 2x mode), then per-entity PE
    ones-matmuls accumulate gpp[rows, e] = sum_d(t2h_e + t2k_e) in PSUM;
    ACT sigmoid -> g16 [rows, E] fp16 (compact).
  - Matmuls per entity: ps_e[rows, f] = hT_e^T@U + kT_e^T@V + sT^T@W in
    PSUM; ACT tanh evac -> h~ row-major fp16.
  - Update: m = g_b*h~ (Pool TT; inner-dim broadcast is free there),
    u = m + h (DVE TT).
  - Norm: u2 = u*u (Pool TT), nsq = DVE tensor_reduce(X) -> [rows, E] f32;
    per-pair epilogue: bit-trick rsqrt + 1 Newton step (DVE smalls; avoids
    ACT table thrash) -> y fp32, y16 fp16.
  - Scale: o = u*y_b split across DVE / Pool / ACT (ACT per-entity Copy
    with per-partition scale=y) ; store one SWDGE DMA per 256-row pair,
    fp16 -> fp32 upcast.

Engine busy time sits well below the ~90us HBM roofline so the DMA wall
dominates.
"""

import numpy as np
from contextlib import nullcontext as _nullctx

B, E, D = 8192, 20, 128
N_CORES = 8
B_LOC = B // N_CORES
CHUNK = 128
N_PAIRS = B_LOC // (2 * CHUNK)
EG = 4  # entities per psum group (4*128 fp32 = one 2KB PSUM bank)
NG = E // EG

_CACHE = {}


def _build_nc(reps=1, loop_n=None, ablate='full',
              io_bufs=3, tr_bufs=2, trs_bufs=2, tt_bufs=2, bf_bufs=2,
              u2_bufs=2, sm_bufs=4, o_bufs=2,
              psm_bufs=3, psk_bufs=2, psg_bufs=2,
              m_eng='gps', u2_eng='dve', u_eng='gps', norm_mode='reduce',
              scale_dve_e=0, scale_gps_e=16,
              s_pe=True, split_first=1, store_pair=True, newton_iters=1,
              dma_scratch=32768, tr_mode='pe', tg_size=8, evac_cycle='addada',
              prefetch=2):
    import concourse.tile as tile
    from concourse import bacc, mybir
    from concourse.masks import make_identity
    from contextlib import ExitStack

    fp32 = mybir.dt.float32
    fp16 = mybir.dt.float16
    int32 = mybir.dt.int32
    AF = mybir.ActivationFunctionType
    OP = mybir.AluOpType
    AX = mybir.AxisListType

    nc = bacc.Bacc("TRN2", target_bir_lowering=False, debug=False,
                   dynamic_dma_scratch_size=dma_scratch)
    enc_d = nc.declare_dram_parameter("enc", [B_LOC, D], fp32, isOutput=False)
    prev_d = nc.declare_dram_parameter("prev", [B_LOC, E, D], fp32, isOutput=False)
    keys_d = nc.declare_dram_parameter("keys", [B_LOC, E, D], fp32, isOutput=False)
    u_d = nc.declare_dram_parameter("U", [D, D], fp32, isOutput=False)
    v_d = nc.declare_dram_parameter("V", [D, D], fp32, isOutput=False)
    w_d = nc.declare_dram_parameter("W", [D, D], fp32, isOutput=False)
    out_d = nc.declare_dram_parameter("out", [B_LOC, E, D], fp32, isOutput=True)

    prev_v = prev_d[:].rearrange("(n two p) e d -> n p two (e d)", two=2, p=CHUNK)
    keys_v = keys_d[:].rearrange("(n two p) e d -> n p two (e d)", two=2, p=CHUNK)
    enc_v = enc_d[:].rearrange("(n two p) d -> n p two d", two=2, p=CHUNK)
    out_pv = out_d[:].rearrange("(n two p) e d -> n p two (e d)", two=2, p=CHUNK)
    out_cv = out_d[:].rearrange("(n p) e d -> n p (e d)", p=CHUNK)

    sd, sp = scale_dve_e, scale_gps_e  # entity split for the final scale

    with ExitStack() as ctx:
        tc = ctx.enter_context(tile.TileContext(nc))
        const_pool = ctx.enter_context(tc.tile_pool(name="const", bufs=1))
        io_pool = ctx.enter_context(tc.tile_pool(name="io", bufs=io_bufs))
        tr_pool = ctx.enter_context(tc.tile_pool(name="tr", bufs=tr_bufs))
        trs_pool = ctx.enter_context(tc.tile_pool(name="trs", bufs=trs_bufs))
        tt_pool = ctx.enter_context(tc.tile_pool(name="tt", bufs=tt_bufs))
        bf_pool = ctx.enter_context(tc.tile_pool(name="bf", bufs=bf_bufs))
        u2_pool = ctx.enter_context(tc.tile_pool(name="u2", bufs=u2_bufs))
        sm_pool = ctx.enter_context(tc.tile_pool(name="sm", bufs=sm_bufs))
        o_pool = ctx.enter_context(tc.tile_pool(name="o", bufs=o_bufs))
        psm_pool = ctx.enter_context(
            tc.tile_pool(name="psm", bufs=psm_bufs, space="PSUM"))
        psk_pool = ctx.enter_context(
            tc.tile_pool(name="psk", bufs=psk_bufs, space="PSUM"))
        psg_pool = ctx.enter_context(
            tc.tile_pool(name="psg", bufs=psg_bufs, space="PSUM"))

        # ---- constants ----
        u32c = const_pool.tile([D, D], fp32)
        v32c = const_pool.tile([D, D], fp32)
        w32c = const_pool.tile([D, D], fp32)
        nc.sync.dma_start(u32c[:], u_d[:])
        nc.sync.dma_start(v32c[:], v_d[:])
        nc.sync.dma_start(w32c[:], w_d[:])
        u16c = const_pool.tile([D, D], fp16)
        v16c = const_pool.tile([D, D], fp16)
        w16c = const_pool.tile([D, D], fp16)
        nc.scalar.copy(u16c[:], u32c[:])
        nc.scalar.copy(v16c[:], v32c[:])
        nc.scalar.copy(w16c[:], w32c[:])
        ones1 = const_pool.tile([D, 1], fp16)
        nc.vector.memset(ones1[:], 1.0)
        magic2 = const_pool.tile([CHUNK, 2, E], int32)
        nc.vector.memset(magic2[:], 0x5F3759DF)
        ident16 = const_pool.tile([D, D], fp16)

        total = N_PAIRS * reps

        def issue_loads(cp, split):
            # paired loads (256 rows), SWDGE cast fp32 -> fp16
            n = cp % N_PAIRS
            h16p = io_pool.tile([CHUNK, 2, E, D], fp16, name="h16p")
            k16p = io_pool.tile([CHUNK, 2, E, D], fp16, name="k16p")
            s16p = io_pool.tile([CHUNK, 2, D], fp16, name="s16p")
            if split:
                # entity-split first chunk so the first transposes start
                # after a quarter of the pair load
                eh = E // 2
                for t_, v_ in ((h16p, prev_v), (k16p, keys_v)):
                    nc.gpsimd.dma_start(
                        t_[:, 0, :eh].rearrange("p e d -> p (e d)"),
                        v_[n][:, 0, : eh * D])
                    nc.gpsimd.dma_start(
                        t_[:, 0, eh:].rearrange("p e d -> p (e d)"),
                        v_[n][:, 0, eh * D :])
                    nc.gpsimd.dma_start(
                        t_[:, 1].rearrange("p e d -> p (e d)"),
                        v_[n][:, 1])
            else:
                nc.gpsimd.dma_start(
                    h16p[:].rearrange("p a e d -> p a (e d)"), prev_v[n])
                nc.gpsimd.dma_start(
                    k16p[:].rearrange("p a e d -> p a (e d)"), keys_v[n])
            nc.gpsimd.dma_start(s16p[:], enc_v[n])
            return h16p, k16p, s16p

        PF = max(0, min(int(prefetch), total - 1))
        pend = {}

        def prologue():
            for i in range(min(PF, total)):
                pend[i] = issue_loads(
                    i, split=(bool(split_first) and i == 0))

        if loop_n is not None:
            make_identity(nc, ident16[:])
        else:
            prologue()
            make_identity(nc, ident16[:])
        loop_cm = (
            tc.For_i(0, loop_n, 1, hint_engines=tuple(mybir.ALL_ENGINES))
            if loop_n is not None
            else _nullctx()
        )
        with loop_cm:
         if loop_n is not None:
            prologue()
         for cp in range(total):
            n = cp % N_PAIRS
            if cp + PF < total:
                pend[cp + PF] = issue_loads(cp + PF, False)
            if PF:
                h16p, k16p, s16p = pend.pop(cp)
            else:
                h16p, k16p, s16p = issue_loads(cp, bool(split_first) and cp == 0)

            if ablate == 'dma':
                hb = h16p[:].rearrange("p a e d -> p a (e d)").bitcast(fp32)
                kb = k16p[:].rearrange("p a e d -> p a (e d)").bitcast(fp32)
                nc.gpsimd.dma_start(out=out_pv[n][:, :, : E * D // 2], in_=hb)
                nc.gpsimd.dma_start(out=out_pv[n][:, :, E * D // 2 :], in_=kb)
                continue

            if ablate == 'xpose':
                for half in range(2):
                    hT = tr_pool.tile([D, E, CHUNK], fp16, name="hT")
                    kT = tr_pool.tile([D, E, CHUNK], fp16, name="kT")
                    nc.sync.dma_start(out=hT[:], in_=h16p[:, half],
                                      transpose=True)
                    nc.sync.dma_start(out=kT[:], in_=k16p[:, half],
                                      transpose=True)
                    nc.gpsimd.dma_start(
                        out=out_cv[2 * n + half][:, : E * D // 2],
                        in_=hT[:].rearrange("p e d -> p (e d)").bitcast(fp32))
                    nc.gpsimd.dma_start(
                        out=out_cv[2 * n + half][:, E * D // 2 :],
                        in_=kT[:].rearrange("p e d -> p (e d)").bitcast(fp32))
                continue

            nsq = sm_pool.tile([CHUNK, 2, E], fp32, name="nsq")
            o16p = o_pool.tile([CHUNK, 2, E, D], fp16, name="o16p")
            for half in range(2):
                c = 2 * n + half
                h16 = h16p[:, half]
                k16 = k16p[:, half]
                s16 = s16p[:, half]

                # ---- forward transposes: PE transpose groups + evacs split
                # across ACT/DVE/Pool (xbar DMA transposes are ~7us each on
                # real HW — avoid them entirely); s via tiny PE + ACT evac ----
                hT = tr_pool.tile([D, E, CHUNK], fp16, name="hT")
                kT = tr_pool.tile([D, E, CHUNK], fp16, name="kT")
                sT = trs_pool.tile([D, CHUNK], fp16, name="sT")
                if s_pe:
                    stp = psk_pool.tile([D, CHUNK], fp16, name="stp", tag="tp")
                    nc.tensor.transpose(stp[:], s16, ident16[:])
                    nc.scalar.copy(sT[:], stp[:])
                else:
                    nc.sync.dma_start(out=sT[:], in_=s16, transpose=True)
                if tr_mode == 'xbar':
                    nc.sync.dma_start(out=hT[:], in_=h16, transpose=True)
                    nc.sync.dma_start(out=kT[:], in_=k16, transpose=True)
                else:
                    tg_bounds = []
                    b0 = 0
                    while b0 < E:
                        tg_bounds.append((b0, min(b0 + tg_size, E)))
                        b0 += tg_size
                    ei = 0
                    for src, dst in ((h16, hT), (k16, kT)):
                        for lo, hi in tg_bounds:
                            tp = psk_pool.tile([D, tg_size, CHUNK], fp16,
                                               name="tp", tag="tp")
                            for j in range(hi - lo):
                                nc.tensor.transpose(
                                    tp[:, j], src[:, lo + j], ident16[:])
                            ev = evac_cycle[ei % len(evac_cycle)]
                            ei += 1
                            if ev == 'a':
                                nc.scalar.copy(dst[:, lo:hi], tp[:, : hi - lo])
                            elif ev == 'p':
                                nc.gpsimd.tensor_copy(
                                    dst[:, lo:hi], tp[:, : hi - lo])
                            else:
                                nc.vector.tensor_copy(
                                    dst[:, lo:hi], tp[:, : hi - lo])

                # ---- gates: t2h/t2k DVE TTs (2x mode: middle-dim bcast),
                # per-entity PE ones-reduce -> compact sigmoid ----
                sTb = sT[:].unsqueeze(1).broadcast_to([D, E, CHUNK])
                t2h = tt_pool.tile([D, E, CHUNK], fp16, name="t2h")
                t2k = tt_pool.tile([D, E, CHUNK], fp16, name="t2k")
                nc.vector.tensor_tensor(t2h[:], hT[:], sTb, OP.mult)
                nc.vector.tensor_tensor(t2k[:], kT[:], sTb, OP.mult)
                gpp = psg_pool.tile([CHUNK, E], fp32, name="gpp")
                for e in range(E):
                    nc.tensor.matmul(
                        gpp[:, e : e + 1], t2h[:, e], ones1[:],
                        start=True, stop=False)
                    nc.tensor.matmul(
                        gpp[:, e : e + 1], t2k[:, e], ones1[:],
                        start=False, stop=True)
                g16 = sm_pool.tile([CHUNK, E], fp16, name="g16")
                nc.scalar.activation(g16[:], gpp[:], AF.Sigmoid)

                # ---- matmuls (row-major psum) + tanh ----
                htil = bf_pool.tile([CHUNK, E, D], fp16, name="htil")
                for gi in range(NG):
                    ps = psm_pool.tile([CHUNK, EG, D], fp32, name="ps")
                    for j in range(EG):
                        e = gi * EG + j
                        nc.tensor.matmul(
                            ps[:, j], hT[:, e], u16c[:],
                            start=True, stop=False)
                        nc.tensor.matmul(
                            ps[:, j], kT[:, e], v16c[:],
                            start=False, stop=False)
                        nc.tensor.matmul(
                            ps[:, j], sT[:], w16c[:],
                            start=False, stop=True)
                    nc.scalar.activation(
                        htil[:, gi * EG : (gi + 1) * EG], ps[:], AF.Tanh)

                # ---- update: u = h + g*h~ (m on Pool, add on DVE) ----
                gb = g16[:].unsqueeze(2).broadcast_to([CHUNK, E, D])
                m_ = htil  # in-place over the tanh output
                if m_eng == 'gps':
                    nc.gpsimd.tensor_tensor(m_[:], htil[:], gb, OP.mult)
                else:
                    nc.vector.tensor_tensor(m_[:], htil[:], gb, OP.mult)
                u16 = bf_pool.tile([CHUNK, E, D], fp16, name="u16")
                if u_eng == 'dve':
                    nc.vector.tensor_tensor(u16[:], m_[:], h16, OP.add)
                else:
                    nc.gpsimd.tensor_tensor(u16[:], m_[:], h16, OP.add)

                # ---- norm: nsq[:, half] = sum_d u^2 ----
                u2 = u2_pool.tile([CHUNK, E, D], fp16, name="u2")
                if norm_mode == 'ttr':
                    # fused square+reduce per entity (one DVE pass)
                    for e in range(E):
                        nc.vector.tensor_tensor_reduce(
                            u2[:, e], u16[:, e], u16[:, e], 1.0, 0.0,
                            OP.mult, OP.add, nsq[:, half, e : e + 1])
                else:
                    if u2_eng == 'gps':
                        nc.gpsimd.tensor_tensor(u2[:], u16[:], u16[:], OP.mult)
                    else:
                        nc.vector.tensor_tensor(u2[:], u16[:], u16[:], OP.mult)
                    nc.vector.tensor_reduce(nsq[:, half], u2[:], AX.X, OP.add)

                if half == 0:
                    u16_prev = u16
                    continue

                # ---- pair epilogue: y = rsqrt(nsq), bit-trick + Newton ----
                ti = sm_pool.tile([CHUNK, 2, E], int32, name="ti")
                nc.vector.tensor_scalar(
                    ti[:], nsq[:].bitcast(int32), 1, None,
                    op0=OP.logical_shift_right)
                yi = sm_pool.tile([CHUNK, 2, E], int32, name="yi")
                nc.vector.tensor_tensor(yi[:], magic2[:], ti[:], OP.subtract)
                y = yi[:].bitcast(fp32)
                for _ in range(newton_iters):
                    y2 = sm_pool.tile([CHUNK, 2, E], fp32, name="y2")
                    nc.vector.tensor_tensor(y2[:], y, y, OP.mult)
                    tt_ = sm_pool.tile([CHUNK, 2, E], fp32, name="tt_")
                    nc.vector.tensor_tensor(tt_[:], nsq[:], y2[:], OP.mult)
                    ww = sm_pool.tile([CHUNK, 2, E], fp32, name="ww")
                    nc.vector.tensor_scalar(
                        ww[:], tt_[:], -0.5, 1.5, op0=OP.mult, op1=OP.add)
                    yn = sm_pool.tile([CHUNK, 2, E], fp32, name="yn")
                    nc.vector.tensor_tensor(yn[:], y, ww[:], OP.mult)
                    y = yn[:]
                y16 = sm_pool.tile([CHUNK, 2, E], fp16, name="y16")
                nc.scalar.copy(y16[:], y)

                # ---- scale o = u*y_b (split DVE / Pool / ACT) + pair store
                for hh, u_c in ((0, u16_prev), (1, u16)):
                    if sd > 0:
                        yb = (y16[:, hh, :sd].unsqueeze(2)
                              .broadcast_to([CHUNK, sd, D]))
                        nc.vector.tensor_tensor(
                            o16p[:, hh, :sd], u_c[:, :sd], yb, OP.mult)
                    if sp > 0:
                        yb = (y16[:, hh, sd : sd + sp].unsqueeze(2)
                              .broadcast_to([CHUNK, sp, D]))
                        nc.gpsimd.tensor_tensor(
                            o16p[:, hh, sd : sd + sp], u_c[:, sd : sd + sp],
                            yb, OP.mult)
                    for e in range(sd + sp, E):
                        nc.scalar.activation(
                            o16p[:, hh, e], u_c[:, e], AF.Copy,
                            scale=y[:, hh, e : e + 1])
                if store_pair:
                    nc.gpsimd.dma_start(
                        out=out_pv[n],
                        in_=o16p[:].rearrange("p a e d -> p a (e d)"))
                else:
                    for hh in range(2):
                        nc.gpsimd.dma_start(
                            out=out_cv[2 * n + hh],
                            in_=o16p[:, hh].rearrange("p e d -> p (e d)"))

    nc.compile()
    return nc


def _get_nc():
    if "nc" not in _CACHE:
        _CACHE["nc"] = _build_nc()
    return _CACHE["nc"]


def kernel(encoded_sents, prev_states, keys, U, V, W):
    import sys

    if "/opt/trn_rl_repo" not in sys.path:
        sys.path.insert(0, "/opt/trn_rl_repo")
    from concourse.bass_utils import run_bass_kernel_spmd

    nc = _get_nc()
    enc = np.ascontiguousarray(np.asarray(encoded_sents, dtype=np.float32))
    prev = np.ascontiguousarray(np.asarray(prev_states, dtype=np.float32))
    kys = np.ascontiguousarray(np.asarray(keys, dtype=np.float32))
    U = np.ascontiguousarray(np.asarray(U, dtype=np.float32))
    V = np.ascontiguousarray(np.asarray(V, dtype=np.float32))
    W = np.ascontiguousarray(np.asarray(W, dtype=np.float32))

    in_maps = []
    for i in range(N_CORES):
        lo, hi = i * B_LOC, (i + 1) * B_LOC
        in_maps.append(
            {
                "enc": enc[lo:hi],
                "prev": prev[lo:hi],
                "keys": kys[lo:hi],
                "U": U,
                "V": V,
                "W": W,
            }
        )

    res = run_bass_kernel_spmd(nc, in_maps, list(range(N_CORES)))
    out = np.concatenate([res.results[i]["out"] for i in range(N_CORES)], axis=0)
    return out.astype(np.float32)


# revision 16
# speedup vs baseline: 1.4176x; 1.2125x over previous
"""Trainium2 Bass kernel for nn_EntityCell (scatter_memory).

Math (per batch row b, entity e):
    gates = sigmoid(sum_d(s * (h + k)))              [B, E]
    h_tilda = tanh(h @ U + k @ V + (s @ W)[:, None]) [B, E, D]
    updated = h + gates[:, :, None] * h_tilda
    out = updated / sqrt(max(sum_d(updated^2), 1e-12))

Sharding: pure data parallel over the batch dim across 8 NeuronCores.

Per-core dataflow (B_loc=1024 rows; 4 load-pairs of 256 rows, each processed
as two 128-row compute chunks). The whole pipeline runs in the TRANSPOSED
(d-major) domain so every elementwise stage is a single full-width VectorE
op instead of 20 per-entity ops:

  - SWDGE (gpsimd) DMA loads cast fp32 HBM -> fp16 SBUF row-major.
  - hT/kT via PE transposes (fp16 PSUM) + ScalarE evac; sT via tiny xbar.
  - Main matmuls WEIGHTS-STATIONARY: psT[f, (e,r)] = U^T hT + V^T kT
    + W^T sT per 4-entity group (lhsT = U/V/W row-major as loaded).
  - ScalarE tanh evacuates psT -> h~T fp16.
  - Gates: c2T = hT+kT, t2T = c2T*sT (two full-width DVE TTs); PE with an
    all-ones stationary matrix broadcasts column sums -> gate presum
    [128,(e,r)] PSUM; ScalarE sigmoid -> G16T fp16.
  - Update: uT = hT + G16T*h~T (two full-width DVE TTs).
  - Norm: u2T = uT*uT (DVE TT); per-entity PE ones-reduce -> nsq [r, e]
    compact; small-tile DVE epilogue (bit-trick rsqrt + Newton) -> y [r,e].
  - Transpose back: o16pre = xbar(uT) row-major; R16 = GpSimd broadcast of
    y along d; o16 = o16pre*R16 (DVE TT); SWDGE store upcasts fp16->fp32.
"""

import numpy as np
from contextlib import nullcontext as _nullctx

B, E, D = 8192, 20, 128
N_CORES = 8
B_LOC = B // N_CORES
CHUNK = 128
N_PAIRS = B_LOC // (2 * CHUNK)
EG = 4  # entities per psum group (4*128 fp32 = one 2KB PSUM bank)
NG = E // EG

_CACHE = {}


def _build_nc(reps=1, loop_n=None, ablate='full',
              io_bufs=3, tr_bufs=4, trs_bufs=2, bf_bufs=4, bfs_bufs=2,
              sm_bufs=4, o_bufs=3,
              psm_bufs=2, psgb_bufs=2, psk_bufs=3, psg_bufs=1,
              tr_mode='pe', tr_evac='split4', back_mode='xbar', s_bcast=True,
              newton_iters=1, scale_mode='mix', scale_tail_dve=1,
              hi_pri_loads=False, split_first=1, s_pe=True,
              u2_gps=0, tg_size=8, all_split=False,
              store_halves=False, mix_eh=10, gb8=False):
    import concourse.tile as tile
    from concourse import bacc, mybir
    from concourse.masks import make_identity
    from contextlib import ExitStack

    fp32 = mybir.dt.float32
    fp16 = mybir.dt.float16
    int32 = mybir.dt.int32
    AF = mybir.ActivationFunctionType
    OP = mybir.AluOpType

    nc = bacc.Bacc("TRN2", target_bir_lowering=False, debug=False)
    enc_d = nc.declare_dram_parameter("enc", [B_LOC, D], fp32, isOutput=False)
    prev_d = nc.declare_dram_parameter("prev", [B_LOC, E, D], fp32, isOutput=False)
    keys_d = nc.declare_dram_parameter("keys", [B_LOC, E, D], fp32, isOutput=False)
    u_d = nc.declare_dram_parameter("U", [D, D], fp32, isOutput=False)
    v_d = nc.declare_dram_parameter("V", [D, D], fp32, isOutput=False)
    w_d = nc.declare_dram_parameter("W", [D, D], fp32, isOutput=False)
    out_d = nc.declare_dram_parameter("out", [B_LOC, E, D], fp32, isOutput=True)

    prev_v = prev_d[:].rearrange("(n two p) e d -> n p two (e d)", two=2, p=CHUNK)
    keys_v = keys_d[:].rearrange("(n two p) e d -> n p two (e d)", two=2, p=CHUNK)
    enc_v = enc_d[:].rearrange("(n two p) d -> n p two d", two=2, p=CHUNK)
    out_v = out_d[:].rearrange("(n p) e d -> n p (e d)", p=CHUNK)

    with ExitStack() as ctx:
        tc = ctx.enter_context(tile.TileContext(nc))
        const_pool = ctx.enter_context(tc.tile_pool(name="const", bufs=1))
        io_pool = ctx.enter_context(tc.tile_pool(name="io", bufs=io_bufs))
        tr_pool = ctx.enter_context(tc.tile_pool(name="tr", bufs=tr_bufs))
        trs_pool = ctx.enter_context(tc.tile_pool(name="trs", bufs=trs_bufs))
        bf_pool = ctx.enter_context(tc.tile_pool(name="bf", bufs=bf_bufs))
        bfs_pool = ctx.enter_context(tc.tile_pool(name="bfs", bufs=bfs_bufs))
        sm_pool = ctx.enter_context(tc.tile_pool(name="sm", bufs=sm_bufs))
        o_pool = ctx.enter_context(tc.tile_pool(name="o", bufs=o_bufs))
        # PSUM: psm = main matmul groups, psgb = gate broadcast groups,
        # psk = PE-transpose staging, psg = compact nsq
        psm_pool = ctx.enter_context(
            tc.tile_pool(name="psm", bufs=psm_bufs, space="PSUM"))
        psgb_pool = ctx.enter_context(
            tc.tile_pool(name="psgb", bufs=psgb_bufs, space="PSUM"))
        psk_pool = ctx.enter_context(
            tc.tile_pool(name="psk", bufs=psk_bufs, space="PSUM"))
        psg_pool = ctx.enter_context(
            tc.tile_pool(name="psg", bufs=psg_bufs, space="PSUM"))

        # ---- constants ----
        u32c = const_pool.tile([D, D], fp32)
        v32c = const_pool.tile([D, D], fp32)
        w32c = const_pool.tile([D, D], fp32)
        nc.sync.dma_start(u32c[:], u_d[:])
        nc.sync.dma_start(v32c[:], v_d[:])
        nc.sync.dma_start(w32c[:], w_d[:])
        u16c = const_pool.tile([D, D], fp16)
        v16c = const_pool.tile([D, D], fp16)
        w16c = const_pool.tile([D, D], fp16)
        nc.scalar.copy(u16c[:], u32c[:])
        nc.scalar.copy(v16c[:], v32c[:])
        nc.scalar.copy(w16c[:], w32c[:])
        ones1 = const_pool.tile([D, 1], fp16)
        nc.vector.memset(ones1[:], 1.0)
        ones128 = const_pool.tile([D, D], fp16)
        nc.vector.memset(ones128[:], 1.0)
        magic2 = const_pool.tile([CHUNK, 2, E], int32)
        nc.vector.memset(magic2[:], 0x5F3759DF)
        ident16 = const_pool.tile([D, D], fp16)

        if loop_n is not None:
            make_identity(nc, ident16[:])
        loop_cm = (
            tc.For_i(0, loop_n, 1, hint_engines=tuple(mybir.ALL_ENGINES))
            if loop_n is not None
            else _nullctx()
        )
        ident_made = loop_n is not None
        with loop_cm:
         for cp in range(N_PAIRS * reps):
            n = cp % N_PAIRS
            if n == 0:
                nsq_all = psg_pool.tile([CHUNK, 2 * N_PAIRS, E], fp32,
                                        name="nsq_all")
            # ---- paired loads (256 rows), SWDGE cast fp32 -> fp16 ----
            h16p = io_pool.tile([CHUNK, 2, E, D], fp16, name="h16p")
            k16p = io_pool.tile([CHUNK, 2, E, D], fp16, name="k16p")
            s16p = io_pool.tile([CHUNK, 2, D], fp16, name="s16p")
            from contextlib import nullcontext
            pri_cm = tc.high_priority() if hi_pri_loads else nullcontext()
            with pri_cm:
                if split_first and cp < int(split_first):
                    eh = E // 2
                    for hh in range(2):
                        if cp == 0 and hh == 0:
                            # entity-split first chunk: first transpose
                            # group starts after a quarter of the pair load
                            for t_, v_ in ((h16p, prev_v), (k16p, keys_v)):
                                nc.gpsimd.dma_start(
                                    t_[:, 0, :eh].rearrange(
                                        "p e d -> p (e d)"),
                                    v_[n][:, 0, : eh * D])
                                nc.gpsimd.dma_start(
                                    t_[:, 0, eh:].rearrange(
                                        "p e d -> p (e d)"),
                                    v_[n][:, 0, eh * D :])
                            continue_hh = True
                        else:
                            nc.gpsimd.dma_start(
                                h16p[:, hh].rearrange("p e d -> p (e d)"),
                                prev_v[n][:, hh])
                            nc.gpsimd.dma_start(
                                k16p[:, hh].rearrange("p e d -> p (e d)"),
                                keys_v[n][:, hh])
                else:
                    nc.gpsimd.dma_start(
                        h16p[:].rearrange("p a e d -> p a (e d)"), prev_v[n])
                    nc.gpsimd.dma_start(
                        k16p[:].rearrange("p a e d -> p a (e d)"), keys_v[n])
                nc.gpsimd.dma_start(s16p[:], enc_v[n])
            if not ident_made:
                make_identity(nc, ident16[:])
                ident_made = True

            if ablate == 'dma':
                for half in range(2):
                    hb = h16p[:, half].rearrange("p e d -> p (e d)").bitcast(fp32)
                    nc.scalar.dma_start(
                        out=out_v[2 * n + half][:, : E * D // 2], in_=hb)
                    nc.scalar.dma_start(
                        out=out_v[2 * n + half][:, E * D // 2 :], in_=hb)
                continue

            for half in range(2):
                c = 2 * n + half
                h16 = h16p[:, half]
                k16 = k16p[:, half]
                s16 = s16p[:, half]

                # ---- sT first (tiny xbar); h/k transposed per group below ----
                hT = tr_pool.tile([D, E, CHUNK], fp16, name="hT")
                kT = tr_pool.tile([D, E, CHUNK], fp16, name="kT")
                sT = tr_pool.tile([D, CHUNK], fp16, name="sT")
                if s_pe:
                    stp = psk_pool.tile([D, CHUNK], fp16, name="stp", tag="tp")
                    nc.tensor.transpose(stp[:], s16, ident16[:])
                    nc.scalar.copy(sT[:], stp[:])
                else:
                    nc.sync.dma_start(out=sT[:], in_=s16, transpose=True)
                if tr_evac == 'act':
                    n_act, act_late = 10, False
                elif tr_evac == 'dve':
                    n_act, act_late = 0, False
                elif tr_evac.startswith('rsplit'):
                    n_act, act_late = int(tr_evac[6:]), True
                else:
                    n_act, act_late = int(tr_evac[5:]), False

                tg_bounds = []
                b0 = 0
                while b0 < E:
                    tg_bounds.append((b0, min(b0 + tg_size, E)))
                    b0 += tg_size
                n_tg = len(tg_bounds)

                def transpose_group(src, dst, gi, ei):
                    lo, hi = tg_bounds[gi]
                    tp = psk_pool.tile([D, tg_size, CHUNK], fp16, name="tp",
                                       tag="tp")
                    for j in range(hi - lo):
                        nc.tensor.transpose(
                            tp[:, j], src[:, lo + j], ident16[:])
                    on_act = (ei >= 6 - n_act) if act_late else (ei < n_act)
                    if on_act:
                        nc.scalar.copy(dst[:, lo:hi], tp[:, : hi - lo])
                    else:
                        nc.vector.tensor_copy(dst[:, lo:hi], tp[:, : hi - lo])

                if tr_mode == 'xbar':
                    nc.sync.dma_start_transpose(out=hT[:], in_=h16)
                    nc.sync.dma_start_transpose(out=kT[:], in_=k16)

                if ablate == 'xpose':
                    nc.scalar.dma_start(
                        out=out_v[c][:, : E * D // 2],
                        in_=hT[:].rearrange("p e d -> p (e d)").bitcast(fp32))
                    nc.scalar.dma_start(
                        out=out_v[c][:, E * D // 2 :],
                        in_=kT[:].rearrange("p e d -> p (e d)").bitcast(fp32))
                    continue

                # ---- per-group pipeline: transpose, gates, matmuls, tanh,
                # sigmoid, update, square, nsq-reduce ----
                c2T = trs_pool.tile([D, E, CHUNK], fp16, name="c2T")
                sTb = sT[:].unsqueeze(1).broadcast_to([D, E, CHUNK])
                t2T = c2T
                htT = bf_pool.tile([D, E, CHUNK], fp16, name="htT")
                G16T = bfs_pool.tile([D, E, CHUNK], fp16, name="G16T")
                u2T = trs_pool.tile([D, E, CHUNK], fp16, name="u2T")
                uT = htT  # in-place over the tanh output
                if tr_mode == 'pe':
                    for ti_, (src, dst) in enumerate(((h16, hT), (k16, kT))):
                        for gi in range(n_tg):
                            transpose_group(src, dst, gi, ti_ * n_tg + gi)
                elif tr_mode == 'mixed':
                    nc.sync.dma_start_transpose(out=hT[:], in_=h16)
                    for gi in range(n_tg):
                        transpose_group(k16, kT, gi, gi)
                nc.vector.tensor_tensor(c2T[:], hT[:], kT[:], OP.add)
                nc.vector.tensor_tensor(t2T[:], c2T[:], sTb, OP.mult)
                for gi in range(NG):
                    sl = slice(gi * EG, (gi + 1) * EG)
                    psT = psm_pool.tile([D, EG, CHUNK], fp32, name="psT")
                    nc.tensor.matmul(
                        psT[:], u16c[:], hT[:, sl],
                        start=True, stop=False)
                    nc.tensor.matmul(
                        psT[:], v16c[:], kT[:, sl],
                        start=False, stop=False)
                    if s_bcast:
                        nc.tensor.matmul(
                            psT[:], w16c[:],
                            sT[:].unsqueeze(1).broadcast_to([D, EG, CHUNK]),
                            start=False, stop=True)
                    else:
                        for j in range(EG):
                            nc.tensor.matmul(
                                psT[:, j], w16c[:], sT[:],
                                start=False, stop=(j == EG - 1))
                    nc.scalar.activation(htT[:, sl], psT[:], AF.Tanh)

                    # gate presum broadcast for this group (ones128 stationary)
                    if gb8:
                        if gi % 2 == 0:
                            gb2 = psgb_pool.tile([D, 2, EG, CHUNK], fp32,
                                                 name="gb2")
                        nc.tensor.matmul(
                            gb2[:, gi % 2], ones128[:], t2T[:, sl],
                            start=True, stop=True)
                        if gi % 2 == 1:
                            nc.scalar.activation(
                                G16T[:, (gi - 1) * EG : (gi + 1) * EG],
                                gb2[:], AF.Sigmoid)
                        elif gi == NG - 1:
                            nc.scalar.activation(
                                G16T[:, sl], gb2[:, 0], AF.Sigmoid)
                    else:
                        gb = psgb_pool.tile([D, EG, CHUNK], fp32, name="gb")
                        nc.tensor.matmul(
                            gb[:], ones128[:], t2T[:, sl],
                            start=True, stop=True)
                        nc.scalar.activation(G16T[:, sl], gb[:], AF.Sigmoid)

                if ablate != 'compute':
                    # update + square (full-width) + per-entity nsq reduce
                    nc.vector.tensor_tensor(uT[:], htT[:], G16T[:], OP.mult)
                    nc.vector.tensor_tensor(uT[:], uT[:], hT[:], OP.add)
                    if u2_gps > 0:
                        nc.gpsimd.tensor_tensor(
                            u2T[:, :u2_gps], uT[:, :u2_gps], uT[:, :u2_gps],
                            OP.mult)
                        nc.vector.tensor_tensor(
                            u2T[:, u2_gps:], uT[:, u2_gps:], uT[:, u2_gps:],
                            OP.mult)
                    else:
                        nc.vector.tensor_tensor(u2T[:], uT[:], uT[:], OP.mult)
                    for e in range(E):
                        nc.tensor.matmul(
                            nsq_all[:, c, e : e + 1], u2T[:, e], ones1[:],
                            start=True, stop=True)

                if ablate == 'compute':
                    nc.scalar.dma_start(
                        out=out_v[c][:, : E * D // 2],
                        in_=htT[:].rearrange("p e d -> p (e d)").bitcast(fp32))
                    nc.scalar.dma_start(
                        out=out_v[c][:, E * D // 2 :],
                        in_=G16T[:].rearrange("p e d -> p (e d)").bitcast(fp32))
                    continue

                # ---- compact rsqrt (both halves of the pair at once):
                # bit-trick seed + Newton ----
                if half == 0:
                    uT_prev = uT
                    continue  # epilogue for both chunks after the second half
                a32 = nsq_all[:, c - 1 : c + 1]  # [CHUNK, 2, E] psum view
                ti = sm_pool.tile([CHUNK, 2, E], int32, name="ti")
                nc.vector.tensor_scalar(
                    ti[:], a32.bitcast(int32), 1, None,
                    op0=OP.logical_shift_right)
                yi = sm_pool.tile([CHUNK, 2, E], int32, name="yi")
                nc.vector.tensor_tensor(yi[:], magic2[:], ti[:], OP.subtract)
                y = yi[:].bitcast(fp32)
                for _ in range(newton_iters):
                    y2 = sm_pool.tile([CHUNK, 2, E], fp32, name="y2")
                    nc.vector.tensor_tensor(y2[:], y, y, OP.mult)
                    tt = sm_pool.tile([CHUNK, 2, E], fp32, name="tt")
                    nc.vector.tensor_tensor(tt[:], a32, y2[:], OP.mult)
                    ww = sm_pool.tile([CHUNK, 2, E], fp32, name="ww")
                    nc.vector.tensor_scalar(
                        ww[:], tt[:], -0.5, 1.5, op0=OP.mult, op1=OP.add)
                    yn = sm_pool.tile([CHUNK, 2, E], fp32, name="yn")
                    nc.vector.tensor_tensor(yn[:], y, ww[:], OP.mult)
                    y = yn[:]

                # ---- transpose back, scale, store (both chunks) ----
                tail = n >= N_PAIRS - scale_tail_dve
                for ci, uT_c in ((c - 1, uT_prev), (c, uT)):
                    o16pre = o_pool.tile([CHUNK, E, D], fp16, name="o16pre")
                    if back_mode == 'xbar':
                        nc.sync.dma_start_transpose(out=o16pre[:], in_=uT_c[:])
                    elif back_mode == 'alt':
                        ring = nc.sync if ci % 2 == 0 else nc.scalar
                        ring.dma_start(out=o16pre[:], in_=uT_c[:],
                                       transpose=True)
                    elif back_mode == 'xbar_act':
                        nc.scalar.dma_start(out=o16pre[:], in_=uT_c[:],
                                            transpose=True)
                    else:
                        for gi in range(NG):
                            tb = psk_pool.tile([CHUNK, EG, D], fp16, name="tb",
                                               tag="tp")
                            for j in range(EG):
                                nc.tensor.transpose(
                                    tb[:, j], uT_c[:, gi * EG + j], ident16[:])
                            nc.scalar.copy(
                                o16pre[:, gi * EG : (gi + 1) * EG], tb[:])
                    # scale by y broadcast along d; engine per scale_mode,
                    # with the last scale_tail_dve pairs on DVE (short tail)
                    yb = (y[:, ci - (c - 1)].unsqueeze(2)
                          .broadcast_to([CHUNK, E, D]))
                    o16 = o16pre  # in-place
                    if scale_mode == 'r16' and not tail:
                        R16 = o_pool.tile([CHUNK, E, D], fp16, name="R16")
                        nc.gpsimd.tensor_copy(R16[:], yb)
                        nc.vector.tensor_tensor(
                            o16[:], o16pre[:], R16[:], OP.mult)
                    elif scale_mode == 'gps' and not tail:
                        nc.gpsimd.tensor_tensor(o16[:], o16pre[:], yb, OP.mult)
                    elif scale_mode == 'mix' and not tail:
                        eh = mix_eh
                        nc.gpsimd.tensor_tensor(
                            o16[:, :eh], o16pre[:, :eh], yb[:, :eh], OP.mult)
                        if store_halves:
                            nc.gpsimd.dma_start(
                                out=out_v[ci][:, : eh * D],
                                in_=o16[:, :eh].rearrange(
                                    "p e d -> p (e d)"))
                        nc.vector.tensor_tensor(
                            o16[:, eh:], o16pre[:, eh:], yb[:, eh:], OP.mult)
                        if store_halves:
                            nc.gpsimd.dma_start(
                                out=out_v[ci][:, eh * D :],
                                in_=o16[:, eh:].rearrange(
                                    "p e d -> p (e d)"))
                            continue
                    elif (all_split or (tail and ci >= 2 * N_PAIRS - 2)) and reps == 1:
                        eh = E // 2
                        nc.vector.tensor_tensor(
                            o16[:, :eh], o16pre[:, :eh], yb[:, :eh], OP.mult)
                        nc.gpsimd.dma_start(
                            out=out_v[ci][:, : eh * D],
                            in_=o16[:, :eh].rearrange("p e d -> p (e d)"))
                        nc.vector.tensor_tensor(
                            o16[:, eh:], o16pre[:, eh:], yb[:, eh:], OP.mult)
                        nc.gpsimd.dma_start(
                            out=out_v[ci][:, eh * D :],
                            in_=o16[:, eh:].rearrange("p e d -> p (e d)"))
                        continue
                    else:
                        nc.vector.tensor_tensor(o16[:], o16pre[:], yb, OP.mult)
                    nc.gpsimd.dma_start(
                        out=out_v[ci], in_=o16[:].rearrange("p e d -> p (e d)"))

    nc.compile()
    return nc


def _get_nc():
    if "nc" not in _CACHE:
        _CACHE["nc"] = _build_nc()
    return _CACHE["nc"]


def kernel(encoded_sents, prev_states, keys, U, V, W):
    import sys

    if "/opt/trn_rl_repo" not in sys.path:
        sys.path.insert(0, "/opt/trn_rl_repo")
    from concourse.bass_utils import run_bass_kernel_spmd

    nc = _get_nc()
    enc = np.ascontiguousarray(np.asarray(encoded_sents, dtype=np.float32))
    prev = np.ascontiguousarray(np.asarray(prev_states, dtype=np.float32))
    kys = np.ascontiguousarray(np.asarray(keys, dtype=np.float32))
    U = np.ascontiguousarray(np.asarray(U, dtype=np.float32))
    V = np.ascontiguousarray(np.asarray(V, dtype=np.float32))
    W = np.ascontiguousarray(np.asarray(W, dtype=np.float32))

    in_maps = []
    for i in range(N_CORES):
        lo, hi = i * B_LOC, (i + 1) * B_LOC
        in_maps.append(
            {
                "enc": enc[lo:hi],
                "prev": prev[lo:hi],
                "keys": kys[lo:hi],
                "U": U,
                "V": V,
                "W": W,
            }
        )

    res = run_bass_kernel_spmd(nc, in_maps, list(range(N_CORES)))
    out = np.concatenate([res.results[i]["out"] for i in range(N_CORES)], axis=0)
    return out.astype(np.float32)

